# revision 10
# baseline (speedup 1.0000x reference)
"""Trainium2 Bass kernel for masked-attention transformer block (v2).

Computes, per batch item b (B=256, S=512, D_IN=256, D_ATT=512):
    Q = x@Wq + bq + pe;  K = x@Wk + bk + pe;  V = x@Wv + bv + pe
    scores = Q K^T / sqrt(D);  scores[:, k >= mask_start[b]] = -inf
    attn = softmax(scores);  o = attn@V + V;  y = LN(o) * gamma + beta
    out = y@Wf + bf + y

Sharding: data-parallel over batch, 32 items per core across 8 cores.

v2 strategy (per item):
  - scores expanded host-side: with Pq = pe + bq, Pk = pe + bk,
      scoresT[k,q] = x A' x^T + x D1 + D2 x^T + C0T
    where A' = Wk Wq^T, D1 = Wk Pq^T, D2T = Wq Pk^T, C0T = Pk Pq^T are
    input-independent. All data matmuls of the scores path run in fp8
    (e4m3) with DoubleRow perf mode (2 rows/cycle) - softmax + the V
    residual attenuate fp8 error to ~1e-4 of the output.
  - numerator attn@V and its denominator also run fp8 DoubleRow with a
    consistent quantized E, so attention rows still sum to exactly 1.
  - x^T comes from a single XBAR DMA transpose per item (x shipped to
    DRAM in bf16 by the host); the transposed rows arrive interleaved
    (row 2p+k at partition p, subtile k) so every constant that
    contracts against x^T is host-permuted to the same interleaved
    order. Same trick for y^T via a DRAM roundtrip + XBAR (wg2 is
    host-permuted to the 4-way interleave), so the PE does zero
    transposes.
  - pe+bias adds fold into the matmuls as an extra identity k-tile
    (psum += I^T @ const) for V; the C0T score term is applied
    multiplicatively after the exp (E = exp(data) * G, G = exp(scale*C0T)
    precomputed) so it runs on the otherwise-idle GpSimd engine (GPSIMD
    cannot touch PSUM on trn2).
  - layernorm row-scale invariance: o'' = den*V + num normalized
    directly; rsqrt computed as exp(-0.5*ln(arg)) so the ACT engine
    never leaves the exp/identity activation-table set (no table swap
    stalls).
  - final matmul stays bf16: out = y@Wg2 + c with Wg2 = diag(g)Wf +
    diag(g) folded host-side.
"""

import numpy as np

import concourse.tile as tile
from concourse import bacc, mybir
from concourse.bass_utils import run_bass_kernel_spmd

N_CORES = 8
B, S, D_IN, D_ATT = 256, 512, 256, 512
BPC = B // N_CORES
EPS = 1e-5
SCALE = float(1.0 / np.sqrt(D_ATT))
NEG = -30000.0
FP32 = mybir.dt.float32
BF16 = mybir.dt.bfloat16
F8 = mybir.dt.float8e4
P = 128
KI = D_IN // P   # 2  k-tiles over input dim
KS = S // P      # 4  tiles over seq
KD = D_ATT // P  # 4  tiles over attention dim
ASCALE = 16.0    # host-side scale on A' to keep fp8 out of denormals

AF = mybir.ActivationFunctionType
OP = mybir.AluOpType
DR = mybir.MatmulPerfMode.DoubleRow

# set by test harness to capture profiling info
TRACE = False
LAST_RESULTS = None


def build_program(n_items, has_cf=False, fin_fp8=False, res_scale=8.0):
    """fin_fp8: final matmul in fp8 DoubleRow with the identity split out.
    Requires cf == 0 and constant gamma (res_scale = 4*gamma0);
    t2 is produced as y/4 so the fp8 quantize needs no scale and
    wg18 = 4*gamma0*Wf compensates exactly."""
    nc = bacc.Bacc(None, target_bir_lowering=False, debug=False)

    xbf_d = nc.dram_tensor("xbf", [n_items, S, D_IN], BF16, kind="ExternalInput")
    m_d = nc.dram_tensor("mstart", [1, n_items], FP32, kind="ExternalInput")
    ap8_d = nc.dram_tensor("ap8", [P, KI, D_IN], F8, kind="ExternalInput")
    d18_d = nc.dram_tensor("d18", [P, KI, S], F8, kind="ExternalInput")
    d2t8_d = nc.dram_tensor("d2t8", [P, KI, S], F8, kind="ExternalInput")
    c0t_d = nc.dram_tensor("c0tb", [P, KS, S], BF16, kind="ExternalInput")
    wv_d = nc.dram_tensor("wvi", [P, KI, D_ATT], BF16, kind="ExternalInput")
    pbv_d = nc.dram_tensor("pbv", [P, KS, D_ATT], BF16, kind="ExternalInput")
    if fin_fp8:
        wg2_d = nc.dram_tensor("wg18", [P, KD, D_ATT], F8, kind="ExternalInput")
    else:
        wg2_d = nc.dram_tensor("wg2i", [P, KD, D_ATT], BF16, kind="ExternalInput")
    cf_d = nc.dram_tensor("cfull", [P, D_ATT], FP32, kind="ExternalInput")
    io_d = nc.dram_tensor("iota4", [P, KS], FP32, kind="ExternalInput")
    id_d = nc.dram_tensor("ident", [P, P], BF16, kind="ExternalInput")
    out_d = nc.dram_tensor("out", [n_items, S, D_ATT], FP32, kind="ExternalOutput")

    with tile.TileContext(nc) as tc:
        with (
            tc.tile_pool(name="const", bufs=1) as cpool,
            tc.tile_pool(name="work", bufs=4) as wpool,
            tc.tile_pool(name="outp", bufs=3) as opool,
            tc.tile_pool(name="small", bufs=4) as spool,
            tc.tile_pool(name="dramp", bufs=3, space="DRAM") as dpool,
            tc.tile_pool(name="psV", bufs=2, space="PSUM") as psV,
            tc.tile_pool(name="psW", bufs=3, space="PSUM") as psW,
            tc.tile_pool(name="psD", bufs=1, space="PSUM") as psD,
        ):
            # ---------------- prefetched x^T for item 0 ----------------
            xT_tiles = {}

            def ensure_xT(b):
                if b in xT_tiles or b >= n_items:
                    return
                t = wpool.tile([P, KI, S], BF16, tag="xT", name=f"xT{b}")
                nc.sync.dma_start(out=t, in_=xbf_d[b], transpose=True)
                xT_tiles[b] = t

            ensure_xT(0)

            # ---------------- constants (loaded once) ----------------
            ident = cpool.tile([P, P], BF16, name="ident_sb")
            nc.sync.dma_start(out=ident, in_=id_d[:])
            ap8 = cpool.tile([P, KI, D_IN], F8, name="ap8_sb")
            nc.sync.dma_start(out=ap8, in_=ap8_d[:])
            d18 = cpool.tile([P, KI, S], F8, name="d18_sb")
            nc.sync.dma_start(out=d18, in_=d18_d[:])
            d2t8 = cpool.tile([P, KI, S], F8, name="d2t8_sb")
            nc.sync.dma_start(out=d2t8, in_=d2t8_d[:])
            c0t = cpool.tile([P, KS, S], BF16, name="c0t_sb")
            nc.sync.dma_start(out=c0t, in_=c0t_d[:])
            wv = cpool.tile([P, KI, D_ATT], BF16, name="wv_sb")
            nc.sync.dma_start(out=wv, in_=wv_d[:])
            pbv = cpool.tile([P, KS, D_ATT], BF16, name="pbv_sb")
            nc.sync.dma_start(out=pbv, in_=pbv_d[:])
            wg2 = cpool.tile([P, KD, D_ATT], F8 if fin_fp8 else BF16, name="wg2_sb")
            nc.sync.dma_start(out=wg2, in_=wg2_d[:])
            magic_t = cpool.tile([P, KS], mybir.dt.uint32, name="magic_t")
            nc.vector.memset(magic_t, 0x5F3759DF)
            cf = cpool.tile([P, D_ATT], FP32, name="cf_sb")
            nc.sync.dma_start(out=cf, in_=cf_d[:])
            iota = cpool.tile([P, KS], FP32, name="iota_sb")
            nc.sync.dma_start(out=iota, in_=io_d[:])

            ones8 = cpool.tile([P, KI, 1], F8, name="ones8")
            nc.vector.memset(ones8, 1.0)
            zero_t = cpool.tile([P, 1], FP32, name="zero_t")
            nc.vector.memset(zero_t, 0.0)

            # broadcast mask starts to all 128 partitions on GpSimd
            m_row = cpool.tile([1, n_items], FP32, name="m_row")
            nc.sync.dma_start(out=m_row, in_=m_d[:])
            m_bc = cpool.tile([P, n_items], FP32, name="m_bc")
            nc.gpsimd.partition_broadcast(m_bc, m_row)

            # ---------------- per-item stages ----------------

            def stageA1(b):
                ensure_xT(b)
                ensure_xT(b + 1)
                xT = xT_tiles.pop(b)
                xT8 = wpool.tile([P, KI, S], F8, tag="xT8", name=f"xT8_{b}")
                nc.scalar.copy(xT8, xT)

                # u^T = A'^T x^T (fp8, scaled down by ASCALE on evict);
                # both psum banks evicted in one ACT op
                uT8 = wpool.tile([P, KI, S], F8, tag="uT8", name=f"uT8_{b}")
                ups = psV.tile([P, KI, S], FP32, tag="vpair")
                for e in range(KI):
                    nc.tensor.matmul(
                        ups[:, e, :],
                        lhsT=ap8[:, 0:KI, P * e : P * (e + 1)],
                        rhs=xT8[:, 0:KI, :],
                        start=True, stop=True, perf_mode=DR,
                    )
                nc.scalar.mul(uT8, ups, 1.0 / ASCALE)

                maskb = spool.tile([P, KS], FP32, tag="maskb", name=f"maskb{b}")
                nc.vector.tensor_scalar(
                    maskb, iota, m_bc[:, b : b + 1], NEG, OP.is_ge, OP.mult
                )

                # scoresT k-tiles (fp8 DR + a bf16 identity k-tile adding the
                # constant C0T term) -> exp -> E^T (fp8)
                ET8 = wpool.tile([P, KS, S], F8, tag="ET8", name=f"ET8_{b}")
                for m in range(KS):
                    ps = psW.tile([P, S], FP32, tag="ps")
                    nc.tensor.matmul(
                        ps, lhsT=uT8[:, 0:KI, P * m : P * (m + 1)],
                        rhs=xT8[:, 0:KI, :],
                        start=True, stop=False, perf_mode=DR,
                    )
                    nc.tensor.matmul(
                        ps, lhsT=d2t8[:, 0:KI, P * m : P * (m + 1)],
                        rhs=xT8[:, 0:KI, :],
                        start=False, stop=False, perf_mode=DR,
                    )
                    nc.tensor.matmul(
                        ps, lhsT=xT8[:, 0:KI, P * m : P * (m + 1)],
                        rhs=d18[:, 0:KI, :],
                        start=False, stop=False, perf_mode=DR,
                    )
                    nc.tensor.matmul(
                        ps, lhsT=ident, rhs=c0t[:, m, :], start=False, stop=True
                    )
                    nc.scalar.activation(
                        out=ET8[:, m, :], in_=ps, func=AF.Exp,
                        bias=maskb[:, m : m + 1], scale=SCALE,
                    )
                return xT, xT8, ET8

            def stageA2(b, xT, xT8, ET8):
                # V projection (psum keeps pe+bv via identity k-tile);
                # evicted twice: fp8 for the numerator matmul, bf16 for the
                # residual (only one PSUM operand allowed per DVE op).
                # Two m-tiles share a psum bank-pair so each evict is one op.
                V8 = wpool.tile([P, KS, D_ATT], F8, tag="V8", name=f"V8_{b}")
                Vbf = wpool.tile([P, KS, D_ATT], BF16, tag="Vbf", name=f"Vbf_{b}")
                for mp in range(0, KS, 2):
                    vps = psV.tile([P, 2, D_ATT], FP32, tag="vpair")
                    for h in range(2):
                        m = mp + h
                        for k in range(KI):
                            nc.tensor.matmul(
                                vps[:, h, :], lhsT=xT[:, k, P * m : P * (m + 1)],
                                rhs=wv[:, k, :],
                                start=(k == 0), stop=False,
                            )
                        nc.tensor.matmul(
                            vps[:, h, :], lhsT=ident, rhs=pbv[:, m, :],
                            start=False, stop=True,
                        )
                    nc.scalar.copy(V8[:, mp : mp + 2, :], vps)
                    nc.vector.tensor_copy(Vbf[:, mp : mp + 2, :], vps)

                # numerator/denominator + residual; layernorm stats
                den4 = spool.tile([P, KS], FP32, tag="den4", name=f"den4_{b}")
                o4 = wpool.tile([P, KS, D_ATT], BF16, tag="o4", name=f"o4_{b}")
                mv4 = spool.tile([P, KS, 2], FP32, tag="mv4", name=f"mv4_{b}")
                for m in range(KS):
                    nps = psW.tile([P, D_ATT], FP32, tag="ps")
                    dps = psD.tile([P, 1], FP32, tag="dps")
                    for t in range(0, KS, 2):
                        nc.tensor.matmul(
                            nps, lhsT=ET8[:, t : t + 2, P * m : P * (m + 1)],
                            rhs=V8[:, t : t + 2, :],
                            start=(t == 0), stop=(t == KS - 2), perf_mode=DR,
                        )
                        nc.tensor.matmul(
                            dps, lhsT=ET8[:, t : t + 2, P * m : P * (m + 1)],
                            rhs=ones8[:, 0:KI, :],
                            start=(t == 0), stop=(t == KS - 2), perf_mode=DR,
                        )
                    nc.vector.tensor_copy(den4[:, m : m + 1], dps)
                    nc.vector.scalar_tensor_tensor(
                        out=o4[:, m, :], in0=Vbf[:, m, :],
                        scalar=den4[:, m : m + 1], in1=nps,
                        op0=OP.mult, op1=OP.add,
                    )
                    stats = spool.tile([P, 6], FP32, tag="stats")
                    nc.vector.bn_stats(stats, o4[:, m, :])
                    nc.vector.bn_aggr(mv4[:, m, :], stats)

                # batched LN scalars; rsqrt via the fast-inverse-sqrt bit
                # trick + one Newton step, all on DVE - the ACT engine then
                # only ever runs Exp/Identity/Copy (one table set, no
                # ACT_TABLE_LOAD swaps). The 0.25 t2-prescale for the fp8
                # final matmul is folded into the Newton constants for free.
                sq4 = spool.tile([P, KS], FP32, tag="sq4", name=f"sq4_{b}")
                nc.vector.tensor_tensor(sq4, den4, den4, op=OP.mult)
                arg4 = spool.tile([P, KS], FP32, tag="arg4", name=f"arg4_{b}")
                nc.vector.scalar_tensor_tensor(
                    out=arg4, in0=sq4, scalar=EPS, in1=mv4[:, :, 1],
                    op0=OP.mult, op1=OP.add,
                )
                h4 = spool.tile([P, KS], mybir.dt.uint32, tag="h4", name=f"h4_{b}")
                nc.vector.tensor_scalar(
                    h4, arg4.bitcast(mybir.dt.uint32), 1, None,
                    OP.logical_shift_right,
                )
                x0u = spool.tile([P, KS], mybir.dt.uint32, tag="x0u", name=f"x0u_{b}")
                nc.vector.tensor_tensor(x0u, magic_t, h4, op=OP.subtract)
                x0 = x0u.bitcast(FP32)
                tt4 = spool.tile([P, KS], FP32, tag="tt4", name=f"tt4_{b}")
                nc.vector.tensor_tensor(tt4, x0, x0, op=OP.mult)
                nc.vector.tensor_tensor(tt4, tt4, arg4, op=OP.mult)
                u4 = spool.tile([P, KS], FP32, tag="u4", name=f"u4_{b}")
                t2s = 0.25 if fin_fp8 else 1.0
                nc.vector.tensor_scalar(
                    u4, tt4, -0.5 * t2s, 1.5 * t2s, OP.mult, OP.add
                )
                rs4 = spool.tile([P, KS], FP32, tag="rs4", name=f"rs4_{b}")
                nc.vector.tensor_tensor(rs4, u4, x0, op=OP.mult)
                nmr4 = spool.tile([P, KS], FP32, tag="nmr4", name=f"nmr4_{b}")
                nc.vector.scalar_tensor_tensor(
                    out=nmr4, in0=mv4[:, :, 0], scalar=-1.0, in1=rs4,
                    op0=OP.mult, op1=OP.mult,
                )

                t2 = wpool.tile([P, KS, D_ATT], BF16, tag="t2", name=f"t2_{b}")
                for m in range(KS):
                    nc.scalar.activation(
                        out=t2[:, m, :], in_=o4[:, m, :], func=AF.Identity,
                        bias=nmr4[:, m : m + 1], scale=rs4[:, m : m + 1],
                    )

                # y^T via DRAM roundtrip + XBAR (interleaved rows, wg2 matches)
                t2d = dpool.tile([S, D_ATT], BF16, tag="t2d", name=f"t2d_{b}")
                nc.sync.dma_start(
                    out=t2d[:].rearrange("(m p) d -> p m d", p=P), in_=t2
                )
                t2T = wpool.tile([P, KD, S], BF16, tag="t2T", name=f"t2T_{b}")
                nc.sync.dma_start(out=t2T, in_=t2d[:], transpose=True)
                if fin_fp8:
                    # GpSimd CAST is slow (~5us) but the engine is idle and
                    # the B(b-2) pipeline hides the latency
                    t2T8 = wpool.tile([P, KD, S], F8, tag="t2T8", name=f"t2T8_{b}")
                    nc.gpsimd.tensor_copy(t2T8, t2T)
                    return t2T8, t2
                return t2T, t2

            def stageB(b, t2T, t2):
                out_sb = opool.tile([P, KS, D_ATT], FP32, tag="osb")
                for mp in range(0, KS, 2):
                    fps = psV.tile([P, 2, D_ATT], FP32, tag="vpair")
                    for h in range(2):
                        m = mp + h
                        if fin_fp8:
                            for t in range(0, KD, 2):
                                nc.tensor.matmul(
                                    fps[:, h, :],
                                    lhsT=t2T[:, t : t + 2, P * m : P * (m + 1)],
                                    rhs=wg2[:, t : t + 2, :],
                                    start=(t == 0), stop=(t == KD - 2),
                                    perf_mode=DR,
                                )
                        else:
                            for t in range(KD):
                                nc.tensor.matmul(
                                    fps[:, h, :], lhsT=t2T[:, t, P * m : P * (m + 1)],
                                    rhs=wg2[:, t, :],
                                    start=(t == 0), stop=(t == KD - 1),
                                )
                    if fin_fp8:
                        # residual g0*y + evict: y = 4*t2 (res_scale = 4*g0)
                        nc.vector.scalar_tensor_tensor(
                            out=out_sb[:, mp : mp + 2, :], in0=t2[:, mp : mp + 2, :],
                            scalar=float(res_scale), in1=fps,
                            op0=OP.mult, op1=OP.add,
                        )
                    elif has_cf:
                        for h in range(2):
                            nc.vector.tensor_add(
                                out_sb[:, mp + h, :], fps[:, h, :], cf
                            )
                    else:
                        nc.vector.tensor_copy(out_sb[:, mp : mp + 2, :], fps)
                nc.sync.dma_start(
                    out=out_d[b].rearrange("(m p) d -> p m d", p=P), in_=out_sb
                )

            # stageB runs two items behind: the t2 -> DRAM -> XBAR -> fp8
            # roundtrip takes ~10us, so the final matmul of item b-2 is the
            # only stage whose operand is guaranteed ready when the PE gets
            # to it.
            held = {}
            heldA = {}
            for b in range(n_items + 2):
                if b < n_items:
                    heldA[b] = stageA1(b)
                if b >= 2:
                    stageB(b - 2, *held.pop(b - 2))
                if b < n_items:
                    held[b] = stageA2(b, *heldA.pop(b))
    nc.compile()
    return nc


def host_consts(Wq, bq, Wk, bk, Wv, bv, Wf, bf, pos_emb, gamma, beta):
    """One-time host-side weight-layout transforms (input-data independent)."""
    import ml_dtypes

    f32 = np.float32
    bf16 = ml_dtypes.bfloat16
    fp8 = ml_dtypes.float8_e4m3

    Wq = np.asarray(Wq, f32)
    Wk = np.asarray(Wk, f32)
    Wv = np.asarray(Wv, f32)
    Wf = np.asarray(Wf, f32)
    pe = np.asarray(pos_emb, f32)[:S]
    gamma = np.asarray(gamma, f32)
    beta = np.asarray(beta, f32)
    Pq = pe + np.asarray(bq, f32)[None, :]
    Pk = pe + np.asarray(bk, f32)[None, :]
    Pv = pe + np.asarray(bv, f32)[None, :]

    # scores expansion constants (see module docstring)
    Ap = (Wk @ Wq.T) * ASCALE                     # [D_IN, D_IN]
    d1 = Wk @ Pq.T                                # [D_IN, S]
    d2t = Wq @ Pk.T                               # [D_IN, S]
    c0t = Pk @ Pq.T                               # [S, S]

    # blocked row order (128k+p) matching the XBAR transpose output of x^T
    def rows2(a):  # [256, n] -> [128, 2, n]
        return np.ascontiguousarray(a.reshape(KI, P, -1).transpose(1, 0, 2))

    ap_i = Ap.reshape(KI, P, D_IN).transpose(1, 0, 2)

    wg2 = gamma[:, None] * Wf + np.diag(gamma).astype(f32)
    c_row = beta @ Wf + np.asarray(bf, f32) + beta

    return {
        "ap8": np.ascontiguousarray(ap_i).astype(fp8),
        "d18": rows2(d1).astype(fp8),
        "d2t8": rows2(d2t).astype(fp8),
        "c0tb": np.ascontiguousarray(
            c0t.reshape(KS, P, S).transpose(1, 0, 2)
        ).astype(bf16),
        "wvi": rows2(Wv).astype(bf16),
        "pbv": np.ascontiguousarray(
            Pv.reshape(KS, P, D_ATT).transpose(1, 0, 2)
        ).astype(bf16),
        # wg2 rows in the blocked (128k+p) order of the t2T XBAR output
        "wg2i": np.ascontiguousarray(
            wg2.reshape(KD, P, D_ATT).transpose(1, 0, 2)
        ).astype(bf16),
        # fp8 final-matmul variant: 4*g0*Wf (identity split out; t2 = y/4)
        "wg18": np.ascontiguousarray(
            (4.0 * float(gamma[0]) * Wf).reshape(KD, P, D_ATT).transpose(1, 0, 2)
        ).astype(fp8),
        "gamma0": float(gamma[0]),
        "gamma_const": bool(np.allclose(gamma, gamma[0], rtol=0, atol=0)),
        "cfull": np.ascontiguousarray(np.broadcast_to(c_row.astype(f32), (P, D_ATT))),
        "iota4": np.ascontiguousarray(
            (np.arange(P, dtype=f32)[:, None] + P * np.arange(KS, dtype=f32)[None, :])
        ),
        "ident": np.eye(P, dtype=f32).astype(bf16),
    }


_prog_cache = {}


def _get_program(n_items, has_cf, fin_fp8, res_scale):
    key = (n_items, has_cf, fin_fp8, res_scale)
    if key not in _prog_cache:
        _prog_cache[key] = build_program(
            n_items, has_cf=has_cf, fin_fp8=fin_fp8, res_scale=res_scale
        )
    return _prog_cache[key]


def kernel(x, mask_start, Wq, bq, Wk, bk, Wv, bv, Wf, bf, pos_emb, gamma, beta):
    global LAST_RESULTS
    import ml_dtypes

    x_bf = np.asarray(x, np.float32).astype(ml_dtypes.bfloat16)
    mask_f = np.asarray(mask_start).astype(np.float32)
    consts = host_consts(Wq, bq, Wk, bk, Wv, bv, Wf, bf, pos_emb, gamma, beta)

    has_cf = bool(np.abs(consts["cfull"]).max() > 1e-8)
    gamma_const = consts.pop("gamma_const")
    gamma0 = consts.pop("gamma0")
    fin_fp8 = (not has_cf) and gamma_const
    res_scale = 4.0 * gamma0
    if fin_fp8:
        consts.pop("wg2i")
    else:
        consts.pop("wg18")
    nc = _get_program(BPC, has_cf, fin_fp8, res_scale)
    in_maps = []
    for c in range(N_CORES):
        m = dict(consts)
        m["xbf"] = np.ascontiguousarray(x_bf[c * BPC : (c + 1) * BPC])
        m["mstart"] = np.ascontiguousarray(mask_f[c * BPC : (c + 1) * BPC])[None, :]
        in_maps.append(m)

    res = run_bass_kernel_spmd(nc, in_maps, core_ids=list(range(N_CORES)), trace=TRACE)
    LAST_RESULTS = res
    out = np.concatenate([res.results[c]["out"] for c in range(N_CORES)], axis=0)
    return out


# revision 11
# speedup vs baseline: 1.0517x; 1.0517x over previous
"""Trainium2 Bass kernel for masked-attention transformer block (v2).

Computes, per batch item b (B=256, S=512, D_IN=256, D_ATT=512):
    Q = x@Wq + bq + pe;  K = x@Wk + bk + pe;  V = x@Wv + bv + pe
    scores = Q K^T / sqrt(D);  scores[:, k >= mask_start[b]] = -inf
    attn = softmax(scores);  o = attn@V + V;  y = LN(o) * gamma + beta
    out = y@Wf + bf + y

Sharding: data-parallel over batch, 32 items per core across 8 cores.

v2 strategy (per item):
  - scores expanded host-side: with Pq = pe + bq, Pk = pe + bk,
      scoresT[k,q] = x A' x^T + x D1 + D2 x^T + C0T
    where A' = Wk Wq^T, D1 = Wk Pq^T, D2T = Wq Pk^T, C0T = Pk Pq^T are
    input-independent. All data matmuls of the scores path run in fp8
    (e4m3) with DoubleRow perf mode (2 rows/cycle) - softmax + the V
    residual attenuate fp8 error to ~1e-4 of the output.
  - numerator attn@V and its denominator also run fp8 DoubleRow with a
    consistent quantized E, so attention rows still sum to exactly 1.
  - x^T comes from a single XBAR DMA transpose per item (x shipped to
    DRAM in bf16 by the host); the transposed rows arrive interleaved
    (row 2p+k at partition p, subtile k) so every constant that
    contracts against x^T is host-permuted to the same interleaved
    order. Same trick for y^T via a DRAM roundtrip + XBAR (wg2 is
    host-permuted to the 4-way interleave), so the PE does zero
    transposes.
  - pe+bias adds fold into the matmuls as an extra identity k-tile
    (psum += I^T @ const) for V; the C0T score term is applied
    multiplicatively after the exp (E = exp(data) * G, G = exp(scale*C0T)
    precomputed) so it runs on the otherwise-idle GpSimd engine (GPSIMD
    cannot touch PSUM on trn2).
  - layernorm row-scale invariance: o'' = den*V + num normalized
    directly; rsqrt computed as exp(-0.5*ln(arg)) so the ACT engine
    never leaves the exp/identity activation-table set (no table swap
    stalls).
  - final matmul stays bf16: out = y@Wg2 + c with Wg2 = diag(g)Wf +
    diag(g) folded host-side.
"""

import numpy as np

import concourse.tile as tile
from concourse import bacc, mybir
from concourse.bass_utils import run_bass_kernel_spmd

N_CORES = 8
B, S, D_IN, D_ATT = 256, 512, 256, 512
BPC = B // N_CORES
EPS = 1e-5
SCALE = float(1.0 / np.sqrt(D_ATT))
NEG = -30000.0
FP32 = mybir.dt.float32
BF16 = mybir.dt.bfloat16
F8 = mybir.dt.float8e4
P = 128
KI = D_IN // P   # 2  k-tiles over input dim
KS = S // P      # 4  tiles over seq
KD = D_ATT // P  # 4  tiles over attention dim
ASCALE = 16.0    # host-side scale on A' to keep fp8 out of denormals

AF = mybir.ActivationFunctionType
OP = mybir.AluOpType
DR = mybir.MatmulPerfMode.DoubleRow

# set by test harness to capture profiling info
TRACE = False
LAST_RESULTS = None


def build_program(n_items, has_cf=False, fin_fp8=False, res_scale=8.0):
    """fin_fp8: final matmul in fp8 DoubleRow with the identity split out.
    Requires cf == 0 and constant gamma (res_scale = 4*gamma0);
    t2 is produced as y/4 so the fp8 quantize needs no scale and
    wg18 = 4*gamma0*Wf compensates exactly."""
    nc = bacc.Bacc(None, target_bir_lowering=False, debug=False)

    xbf_d = nc.dram_tensor("xbf", [n_items, S, D_IN], BF16, kind="ExternalInput")
    m_d = nc.dram_tensor("mstart", [1, n_items], FP32, kind="ExternalInput")
    ap8_d = nc.dram_tensor("ap8", [P, KI, D_IN], F8, kind="ExternalInput")
    d18_d = nc.dram_tensor("d18", [P, KI, S], F8, kind="ExternalInput")
    d2t8_d = nc.dram_tensor("d2t8", [P, KI, S], F8, kind="ExternalInput")
    c0t_d = nc.dram_tensor("c0tb", [P, KS, S], BF16, kind="ExternalInput")
    wv_d = nc.dram_tensor("wvi", [P, KI, D_ATT], BF16, kind="ExternalInput")
    pbv_d = nc.dram_tensor("pbv", [P, KS, D_ATT], BF16, kind="ExternalInput")
    if fin_fp8:
        wg2_d = nc.dram_tensor("wg18", [P, KD, D_ATT], F8, kind="ExternalInput")
    else:
        wg2_d = nc.dram_tensor("wg2i", [P, KD, D_ATT], BF16, kind="ExternalInput")
    cf_d = nc.dram_tensor("cfull", [P, D_ATT], FP32, kind="ExternalInput")
    io_d = nc.dram_tensor("iota4", [P, KS], FP32, kind="ExternalInput")
    id_d = nc.dram_tensor("ident", [P, P], BF16, kind="ExternalInput")
    out_d = nc.dram_tensor("out", [n_items, S, D_ATT], FP32, kind="ExternalOutput")

    with tile.TileContext(nc) as tc:
        with (
            tc.tile_pool(name="const", bufs=1) as cpool,
            tc.tile_pool(name="work", bufs=4) as wpool,
            tc.tile_pool(name="outp", bufs=3) as opool,
            tc.tile_pool(name="small", bufs=4) as spool,
            tc.tile_pool(name="dramp", bufs=3, space="DRAM") as dpool,
            tc.tile_pool(name="psV", bufs=2, space="PSUM") as psV,
            tc.tile_pool(name="psW", bufs=3, space="PSUM") as psW,
            tc.tile_pool(name="psD", bufs=1, space="PSUM") as psD,
        ):
            # ---------------- prefetched x^T for item 0 ----------------
            xT_tiles = {}

            def ensure_xT(b):
                if b in xT_tiles or b >= n_items:
                    return
                t = wpool.tile([P, KI, S], BF16, tag="xT", name=f"xT{b}")
                nc.sync.dma_start(out=t, in_=xbf_d[b], transpose=True)
                xT_tiles[b] = t

            ensure_xT(0)

            # ---------------- constants (loaded once) ----------------
            ident = cpool.tile([P, P], BF16, name="ident_sb")
            nc.sync.dma_start(out=ident, in_=id_d[:])
            ap8 = cpool.tile([P, KI, D_IN], F8, name="ap8_sb")
            nc.sync.dma_start(out=ap8, in_=ap8_d[:])
            d18 = cpool.tile([P, KI, S], F8, name="d18_sb")
            nc.sync.dma_start(out=d18, in_=d18_d[:])
            d2t8 = cpool.tile([P, KI, S], F8, name="d2t8_sb")
            nc.sync.dma_start(out=d2t8, in_=d2t8_d[:])
            c0t = cpool.tile([P, KS, S], BF16, name="c0t_sb")
            nc.sync.dma_start(out=c0t, in_=c0t_d[:])
            wv = cpool.tile([P, KI, D_ATT], BF16, name="wv_sb")
            nc.sync.dma_start(out=wv, in_=wv_d[:])
            pbv = cpool.tile([P, KS, D_ATT], BF16, name="pbv_sb")
            nc.sync.dma_start(out=pbv, in_=pbv_d[:])
            wg2 = cpool.tile([P, KD, D_ATT], F8 if fin_fp8 else BF16, name="wg2_sb")
            nc.sync.dma_start(out=wg2, in_=wg2_d[:])
            magic_t = cpool.tile([P, KS], mybir.dt.uint32, name="magic_t")
            nc.vector.memset(magic_t, 0x5F3759DF)
            cf = cpool.tile([P, D_ATT], FP32, name="cf_sb")
            nc.sync.dma_start(out=cf, in_=cf_d[:])
            iota = cpool.tile([P, KS], FP32, name="iota_sb")
            nc.sync.dma_start(out=iota, in_=io_d[:])

            ones8 = cpool.tile([P, KI, 1], F8, name="ones8")
            nc.vector.memset(ones8, 1.0)
            zero_t = cpool.tile([P, 1], FP32, name="zero_t")
            nc.vector.memset(zero_t, 0.0)

            # broadcast mask starts to all 128 partitions on GpSimd
            m_row = cpool.tile([1, n_items], FP32, name="m_row")
            nc.sync.dma_start(out=m_row, in_=m_d[:])
            m_bc = cpool.tile([P, n_items], FP32, name="m_bc")
            nc.gpsimd.partition_broadcast(m_bc, m_row)

            # ---------------- per-item stages ----------------

            def stageA1(b):
                ensure_xT(b)
                ensure_xT(b + 1)
                xT = xT_tiles.pop(b)
                xT8 = wpool.tile([P, KI, S], F8, tag="xT8", name=f"xT8_{b}")
                nc.scalar.copy(xT8, xT)

                # u^T = A'^T x^T (fp8, scaled down by ASCALE on evict);
                # both psum banks evicted in one ACT op
                uT8 = wpool.tile([P, KI, S], F8, tag="uT8", name=f"uT8_{b}")
                ups = psV.tile([P, KI, S], FP32, tag="vpair")
                for e in range(KI):
                    nc.tensor.matmul(
                        ups[:, e, :],
                        lhsT=ap8[:, 0:KI, P * e : P * (e + 1)],
                        rhs=xT8[:, 0:KI, :],
                        start=True, stop=True, perf_mode=DR,
                    )
                nc.scalar.mul(uT8, ups, 1.0 / ASCALE)

                maskb = spool.tile([P, KS], FP32, tag="maskb", name=f"maskb{b}")
                nc.vector.tensor_scalar(
                    maskb, iota, m_bc[:, b : b + 1], NEG, OP.is_ge, OP.mult
                )

                # scoresT k-tiles (fp8 DR + a bf16 identity k-tile adding the
                # constant C0T term) -> exp -> E^T (fp8). Two m-tiles'
                # accumulation groups are emitted interleaved so each
                # LDWEIGHTS overlaps the other group's matmul stream.
                ET8 = wpool.tile([P, KS, S], F8, tag="ET8", name=f"ET8_{b}")
                for mp in range(0, KS, 2):
                    pss = [
                        psW.tile([P, S], FP32, tag="ps", name=f"ps{b}_{mp}_{h}")
                        for h in range(2)
                    ]
                    for lhs_fn, rhs_fn, st, sp in (
                        (lambda m: uT8[:, 0:KI, P * m : P * (m + 1)],
                         lambda m: xT8[:, 0:KI, :], True, False),
                        (lambda m: d2t8[:, 0:KI, P * m : P * (m + 1)],
                         lambda m: xT8[:, 0:KI, :], False, False),
                        (lambda m: xT8[:, 0:KI, P * m : P * (m + 1)],
                         lambda m: d18[:, 0:KI, :], False, False),
                    ):
                        for h in range(2):
                            nc.tensor.matmul(
                                pss[h], lhsT=lhs_fn(mp + h), rhs=rhs_fn(mp + h),
                                start=st, stop=sp, perf_mode=DR,
                            )
                    for h in range(2):
                        nc.tensor.matmul(
                            pss[h], lhsT=ident, rhs=c0t[:, mp + h, :],
                            start=False, stop=True,
                        )
                    for h in range(2):
                        nc.scalar.activation(
                            out=ET8[:, mp + h, :], in_=pss[h], func=AF.Exp,
                            bias=maskb[:, mp + h : mp + h + 1], scale=SCALE,
                        )
                return xT, xT8, ET8

            def stageA2(b, xT, xT8, ET8):
                # V projection (psum keeps pe+bv via identity k-tile);
                # evicted twice: fp8 for the numerator matmul, bf16 for the
                # residual (only one PSUM operand allowed per DVE op).
                # Two m-tiles share a psum bank-pair so each evict is one op.
                V8 = wpool.tile([P, KS, D_ATT], F8, tag="V8", name=f"V8_{b}")
                Vbf = wpool.tile([P, KS, D_ATT], BF16, tag="Vbf", name=f"Vbf_{b}")
                for mp in range(0, KS, 2):
                    vps = psV.tile([P, 2, D_ATT], FP32, tag="vpair")
                    for k in range(KI):
                        for h in range(2):
                            nc.tensor.matmul(
                                vps[:, h, :],
                                lhsT=xT[:, k, P * (mp + h) : P * (mp + h + 1)],
                                rhs=wv[:, k, :],
                                start=(k == 0), stop=False,
                            )
                    for h in range(2):
                        nc.tensor.matmul(
                            vps[:, h, :], lhsT=ident, rhs=pbv[:, mp + h, :],
                            start=False, stop=True,
                        )
                    nc.scalar.copy(V8[:, mp : mp + 2, :], vps)
                    nc.vector.tensor_copy(Vbf[:, mp : mp + 2, :], vps)

                # numerator/denominator + residual; layernorm stats
                den4 = spool.tile([P, KS], FP32, tag="den4", name=f"den4_{b}")
                o4 = wpool.tile([P, KS, D_ATT], BF16, tag="o4", name=f"o4_{b}")
                mv4 = spool.tile([P, KS, 2], FP32, tag="mv4", name=f"mv4_{b}")
                for m in range(KS):
                    nps = psW.tile([P, D_ATT], FP32, tag="ps")
                    dps = psD.tile([P, 1], FP32, tag="dps")
                    for t in range(0, KS, 2):
                        nc.tensor.matmul(
                            nps, lhsT=ET8[:, t : t + 2, P * m : P * (m + 1)],
                            rhs=V8[:, t : t + 2, :],
                            start=(t == 0), stop=(t == KS - 2), perf_mode=DR,
                        )
                        nc.tensor.matmul(
                            dps, lhsT=ET8[:, t : t + 2, P * m : P * (m + 1)],
                            rhs=ones8[:, 0:KI, :],
                            start=(t == 0), stop=(t == KS - 2), perf_mode=DR,
                        )
                    nc.vector.tensor_copy(den4[:, m : m + 1], dps)
                    nc.vector.scalar_tensor_tensor(
                        out=o4[:, m, :], in0=Vbf[:, m, :],
                        scalar=den4[:, m : m + 1], in1=nps,
                        op0=OP.mult, op1=OP.add,
                    )
                    stats = spool.tile([P, 6], FP32, tag="stats")
                    nc.vector.bn_stats(stats, o4[:, m, :])
                    nc.vector.bn_aggr(mv4[:, m, :], stats)

                # batched LN scalars; rsqrt via the fast-inverse-sqrt bit
                # trick + one Newton step, all on DVE - the ACT engine then
                # only ever runs Exp/Identity/Copy (one table set, no
                # ACT_TABLE_LOAD swaps). The 0.25 t2-prescale for the fp8
                # final matmul is folded into the Newton constants for free.
                sq4 = spool.tile([P, KS], FP32, tag="sq4", name=f"sq4_{b}")
                nc.vector.tensor_tensor(sq4, den4, den4, op=OP.mult)
                arg4 = spool.tile([P, KS], FP32, tag="arg4", name=f"arg4_{b}")
                nc.vector.scalar_tensor_tensor(
                    out=arg4, in0=sq4, scalar=EPS, in1=mv4[:, :, 1],
                    op0=OP.mult, op1=OP.add,
                )
                h4 = spool.tile([P, KS], mybir.dt.uint32, tag="h4", name=f"h4_{b}")
                nc.vector.tensor_scalar(
                    h4, arg4.bitcast(mybir.dt.uint32), 1, None,
                    OP.logical_shift_right,
                )
                x0u = spool.tile([P, KS], mybir.dt.uint32, tag="x0u", name=f"x0u_{b}")
                nc.vector.tensor_tensor(x0u, magic_t, h4, op=OP.subtract)
                x0 = x0u.bitcast(FP32)
                tt4 = spool.tile([P, KS], FP32, tag="tt4", name=f"tt4_{b}")
                nc.vector.tensor_tensor(tt4, x0, x0, op=OP.mult)
                nc.vector.tensor_tensor(tt4, tt4, arg4, op=OP.mult)
                u4 = spool.tile([P, KS], FP32, tag="u4", name=f"u4_{b}")
                t2s = 0.25 if fin_fp8 else 1.0
                nc.vector.tensor_scalar(
                    u4, tt4, -0.5 * t2s, 1.5 * t2s, OP.mult, OP.add
                )
                rs4 = spool.tile([P, KS], FP32, tag="rs4", name=f"rs4_{b}")
                nc.vector.tensor_tensor(rs4, u4, x0, op=OP.mult)
                nmr4 = spool.tile([P, KS], FP32, tag="nmr4", name=f"nmr4_{b}")
                nc.vector.scalar_tensor_tensor(
                    out=nmr4, in0=mv4[:, :, 0], scalar=-1.0, in1=rs4,
                    op0=OP.mult, op1=OP.mult,
                )

                t2 = wpool.tile([P, KS, D_ATT], BF16, tag="t2", name=f"t2_{b}")
                for m in range(KS):
                    nc.scalar.activation(
                        out=t2[:, m, :], in_=o4[:, m, :], func=AF.Identity,
                        bias=nmr4[:, m : m + 1], scale=rs4[:, m : m + 1],
                    )

                # y^T via DRAM roundtrip + XBAR (interleaved rows, wg2 matches)
                t2d = dpool.tile([S, D_ATT], BF16, tag="t2d", name=f"t2d_{b}")
                nc.sync.dma_start(
                    out=t2d[:].rearrange("(m p) d -> p m d", p=P), in_=t2
                )
                t2T = wpool.tile([P, KD, S], BF16, tag="t2T", name=f"t2T_{b}")
                nc.sync.dma_start(out=t2T, in_=t2d[:], transpose=True)
                if fin_fp8:
                    t2T8 = wpool.tile([P, KD, S], F8, tag="t2T8", name=f"t2T8_{b}")
                    nc.scalar.copy(t2T8, t2T)
                    return t2T8, t2
                return t2T, t2

            def stageB(b, t2T, t2):
                out_sb = opool.tile([P, KS, D_ATT], FP32, tag="osb")
                for mp in range(0, KS, 2):
                    fps = psV.tile([P, 2, D_ATT], FP32, tag="vpair")
                    for h in range(2):
                        m = mp + h
                        if fin_fp8:
                            for t in range(0, KD, 2):
                                nc.tensor.matmul(
                                    fps[:, h, :],
                                    lhsT=t2T[:, t : t + 2, P * m : P * (m + 1)],
                                    rhs=wg2[:, t : t + 2, :],
                                    start=(t == 0), stop=(t == KD - 2),
                                    perf_mode=DR,
                                )
                        else:
                            for t in range(KD):
                                nc.tensor.matmul(
                                    fps[:, h, :], lhsT=t2T[:, t, P * m : P * (m + 1)],
                                    rhs=wg2[:, t, :],
                                    start=(t == 0), stop=(t == KD - 1),
                                )
                    if fin_fp8:
                        # residual g0*y + evict: y = 4*t2 (res_scale = 4*g0)
                        nc.vector.scalar_tensor_tensor(
                            out=out_sb[:, mp : mp + 2, :], in0=t2[:, mp : mp + 2, :],
                            scalar=float(res_scale), in1=fps,
                            op0=OP.mult, op1=OP.add,
                        )
                    elif has_cf:
                        for h in range(2):
                            nc.vector.tensor_add(
                                out_sb[:, mp + h, :], fps[:, h, :], cf
                            )
                    else:
                        nc.vector.tensor_copy(out_sb[:, mp : mp + 2, :], fps)
                nc.sync.dma_start(
                    out=out_d[b].rearrange("(m p) d -> p m d", p=P), in_=out_sb
                )

            # stageB runs two items behind: the t2 -> DRAM -> XBAR -> fp8
            # roundtrip takes ~10us, so the final matmul of item b-2 is the
            # only stage whose operand is guaranteed ready when the PE gets
            # to it.
            held = {}
            heldA = {}
            for b in range(n_items + 2):
                if b >= 2:
                    stageB(b - 2, *held.pop(b - 2))
                if b < n_items:
                    heldA[b] = stageA1(b)
                    held[b] = stageA2(b, *heldA.pop(b))
    nc.compile()
    return nc


def host_consts(Wq, bq, Wk, bk, Wv, bv, Wf, bf, pos_emb, gamma, beta):
    """One-time host-side weight-layout transforms (input-data independent)."""
    import ml_dtypes

    f32 = np.float32
    bf16 = ml_dtypes.bfloat16
    fp8 = ml_dtypes.float8_e4m3

    Wq = np.asarray(Wq, f32)
    Wk = np.asarray(Wk, f32)
    Wv = np.asarray(Wv, f32)
    Wf = np.asarray(Wf, f32)
    pe = np.asarray(pos_emb, f32)[:S]
    gamma = np.asarray(gamma, f32)
    beta = np.asarray(beta, f32)
    Pq = pe + np.asarray(bq, f32)[None, :]
    Pk = pe + np.asarray(bk, f32)[None, :]
    Pv = pe + np.asarray(bv, f32)[None, :]

    # scores expansion constants (see module docstring)
    Ap = (Wk @ Wq.T) * ASCALE                     # [D_IN, D_IN]
    d1 = Wk @ Pq.T                                # [D_IN, S]
    d2t = Wq @ Pk.T                               # [D_IN, S]
    c0t = Pk @ Pq.T                               # [S, S]

    # blocked row order (128k+p) matching the XBAR transpose output of x^T
    def rows2(a):  # [256, n] -> [128, 2, n]
        return np.ascontiguousarray(a.reshape(KI, P, -1).transpose(1, 0, 2))

    ap_i = Ap.reshape(KI, P, D_IN).transpose(1, 0, 2)

    wg2 = gamma[:, None] * Wf + np.diag(gamma).astype(f32)
    c_row = beta @ Wf + np.asarray(bf, f32) + beta

    return {
        "ap8": np.ascontiguousarray(ap_i).astype(fp8),
        "d18": rows2(d1).astype(fp8),
        "d2t8": rows2(d2t).astype(fp8),
        "c0tb": np.ascontiguousarray(
            c0t.reshape(KS, P, S).transpose(1, 0, 2)
        ).astype(bf16),
        "wvi": rows2(Wv).astype(bf16),
        "pbv": np.ascontiguousarray(
            Pv.reshape(KS, P, D_ATT).transpose(1, 0, 2)
        ).astype(bf16),
        # wg2 rows in the blocked (128k+p) order of the t2T XBAR output
        "wg2i": np.ascontiguousarray(
            wg2.reshape(KD, P, D_ATT).transpose(1, 0, 2)
        ).astype(bf16),
        # fp8 final-matmul variant: 4*g0*Wf (identity split out; t2 = y/4)
        "wg18": np.ascontiguousarray(
            (4.0 * float(gamma[0]) * Wf).reshape(KD, P, D_ATT).transpose(1, 0, 2)
        ).astype(fp8),
        "gamma0": float(gamma[0]),
        "gamma_const": bool(np.allclose(gamma, gamma[0], rtol=0, atol=0)),
        "cfull": np.ascontiguousarray(np.broadcast_to(c_row.astype(f32), (P, D_ATT))),
        "iota4": np.ascontiguousarray(
            (np.arange(P, dtype=f32)[:, None] + P * np.arange(KS, dtype=f32)[None, :])
        ),
        "ident": np.eye(P, dtype=f32).astype(bf16),
    }


_prog_cache = {}


def _get_program(n_items, has_cf, fin_fp8, res_scale):
    key = (n_items, has_cf, fin_fp8, res_scale)
    if key not in _prog_cache:
        _prog_cache[key] = build_program(
            n_items, has_cf=has_cf, fin_fp8=fin_fp8, res_scale=res_scale
        )
    return _prog_cache[key]


def kernel(x, mask_start, Wq, bq, Wk, bk, Wv, bv, Wf, bf, pos_emb, gamma, beta):
    global LAST_RESULTS
    import ml_dtypes

    x_bf = np.asarray(x, np.float32).astype(ml_dtypes.bfloat16)
    mask_f = np.asarray(mask_start).astype(np.float32)
    consts = host_consts(Wq, bq, Wk, bk, Wv, bv, Wf, bf, pos_emb, gamma, beta)

    has_cf = bool(np.abs(consts["cfull"]).max() > 1e-8)
    gamma_const = consts.pop("gamma_const")
    gamma0 = consts.pop("gamma0")
    fin_fp8 = (not has_cf) and gamma_const
    res_scale = 4.0 * gamma0
    if fin_fp8:
        consts.pop("wg2i")
    else:
        consts.pop("wg18")
    nc = _get_program(BPC, has_cf, fin_fp8, res_scale)
    in_maps = []
    for c in range(N_CORES):
        m = dict(consts)
        m["xbf"] = np.ascontiguousarray(x_bf[c * BPC : (c + 1) * BPC])
        m["mstart"] = np.ascontiguousarray(mask_f[c * BPC : (c + 1) * BPC])[None, :]
        in_maps.append(m)

    res = run_bass_kernel_spmd(nc, in_maps, core_ids=list(range(N_CORES)), trace=TRACE)
    LAST_RESULTS = res
    out = np.concatenate([res.results[c]["out"] for c in range(N_CORES)], axis=0)
    return out


# revision 12
# speedup vs baseline: 1.2984x; 1.2346x over previous
"""Trainium2 Bass kernel for masked-attention transformer block (v2).

Computes, per batch item b (B=256, S=512, D_IN=256, D_ATT=512):
    Q = x@Wq + bq + pe;  K = x@Wk + bk + pe;  V = x@Wv + bv + pe
    scores = Q K^T / sqrt(D);  scores[:, k >= mask_start[b]] = -inf
    attn = softmax(scores);  o = attn@V + V;  y = LN(o) * gamma + beta
    out = y@Wf + bf + y

Sharding: data-parallel over batch, 32 items per core across 8 cores.

v2 strategy (per item):
  - scores expanded host-side: with Pq = pe + bq, Pk = pe + bk,
      scoresT[k,q] = x A' x^T + x D1 + D2 x^T + C0T
    where A' = Wk Wq^T, D1 = Wk Pq^T, D2T = Wq Pk^T, C0T = Pk Pq^T are
    input-independent. All data matmuls of the scores path run in fp8
    (e4m3) with DoubleRow perf mode (2 rows/cycle) - softmax + the V
    residual attenuate fp8 error to ~1e-4 of the output.
  - numerator attn@V and its denominator also run fp8 DoubleRow with a
    consistent quantized E, so attention rows still sum to exactly 1.
  - x^T comes from a single XBAR DMA transpose per item (x shipped to
    DRAM in bf16 by the host); the transposed rows arrive interleaved
    (row 2p+k at partition p, subtile k) so every constant that
    contracts against x^T is host-permuted to the same interleaved
    order. Same trick for y^T via a DRAM roundtrip + XBAR (wg2 is
    host-permuted to the 4-way interleave), so the PE does zero
    transposes.
  - pe+bias adds fold into the matmuls as an extra identity k-tile
    (psum += I^T @ const) for V; the C0T score term is applied
    multiplicatively after the exp (E = exp(data) * G, G = exp(scale*C0T)
    precomputed) so it runs on the otherwise-idle GpSimd engine (GPSIMD
    cannot touch PSUM on trn2).
  - layernorm row-scale invariance: o'' = den*V + num normalized
    directly; rsqrt computed as exp(-0.5*ln(arg)) so the ACT engine
    never leaves the exp/identity activation-table set (no table swap
    stalls).
  - final matmul stays bf16: out = y@Wg2 + c with Wg2 = diag(g)Wf +
    diag(g) folded host-side.
"""

import numpy as np

import concourse.tile as tile
from concourse import bacc, mybir
from concourse.bass_utils import run_bass_kernel_spmd

N_CORES = 8
B, S, D_IN, D_ATT = 256, 512, 256, 512
BPC = B // N_CORES
EPS = 1e-5
SCALE = float(1.0 / np.sqrt(D_ATT))
NEG = -30000.0
FP32 = mybir.dt.float32
BF16 = mybir.dt.bfloat16
F8 = mybir.dt.float8e4
P = 128
KI = D_IN // P   # 2  k-tiles over input dim
KS = S // P      # 4  tiles over seq
KD = D_ATT // P  # 4  tiles over attention dim
ASCALE = 16.0    # host-side scale on A' to keep fp8 out of denormals

AF = mybir.ActivationFunctionType
OP = mybir.AluOpType
DR = mybir.MatmulPerfMode.DoubleRow

# set by test harness to capture profiling info
TRACE = False
LAST_RESULTS = None


def build_program(n_items, has_cf=False, fin_fp8=False, res_scale=8.0):
    """fin_fp8: final matmul in fp8 DoubleRow with the identity split out.
    Requires cf == 0 and constant gamma (res_scale = 4*gamma0);
    t2 is produced as y/4 so the fp8 quantize needs no scale and
    wg18 = 4*gamma0*Wf compensates exactly."""
    nc = bacc.Bacc(None, target_bir_lowering=False, debug=False)

    xbf_d = nc.dram_tensor("xbf", [n_items, S, D_IN], BF16, kind="ExternalInput")
    m_d = nc.dram_tensor("mstart", [1, n_items], FP32, kind="ExternalInput")
    ap8_d = nc.dram_tensor("ap8", [P, KI, D_IN], F8, kind="ExternalInput")
    d18_d = nc.dram_tensor("d18", [P, KI, S], F8, kind="ExternalInput")
    d2t8_d = nc.dram_tensor("d2t8", [P, KI, S], F8, kind="ExternalInput")
    c0t_d = nc.dram_tensor("c0tb", [P, KS, S], BF16, kind="ExternalInput")
    wv_d = nc.dram_tensor("wvi", [P, KI, D_ATT], BF16, kind="ExternalInput")
    pbv_d = nc.dram_tensor("pbv", [P, KS, D_ATT], BF16, kind="ExternalInput")
    if fin_fp8:
        wg2_d = nc.dram_tensor("wg18", [P, KD, D_ATT], F8, kind="ExternalInput")
    else:
        wg2_d = nc.dram_tensor("wg2i", [P, KD, D_ATT], BF16, kind="ExternalInput")
    cf_d = nc.dram_tensor("cfull", [P, D_ATT], FP32, kind="ExternalInput")
    io_d = nc.dram_tensor("iota4", [P, KS], FP32, kind="ExternalInput")
    id_d = nc.dram_tensor("ident", [P, P], BF16, kind="ExternalInput")
    out_d = nc.dram_tensor("out", [n_items, S, D_ATT], FP32, kind="ExternalOutput")

    with tile.TileContext(nc) as tc:
        with (
            tc.tile_pool(name="const", bufs=1) as cpool,
            tc.tile_pool(name="work", bufs=4) as wpool,
            tc.tile_pool(name="outp", bufs=3) as opool,
            tc.tile_pool(name="small", bufs=4) as spool,
            tc.tile_pool(name="dramp", bufs=3, space="DRAM") as dpool,
            tc.tile_pool(name="psV", bufs=2, space="PSUM") as psV,
            tc.tile_pool(name="psW", bufs=3, space="PSUM") as psW,
            tc.tile_pool(name="psD", bufs=1, space="PSUM") as psD,
        ):
            # ---------------- prefetched x^T for item 0 ----------------
            xT_tiles = {}

            def ensure_xT(b):
                if b in xT_tiles or b >= n_items:
                    return
                t = wpool.tile([P, KI, S], BF16, tag="xT", name=f"xT{b}")
                nc.sync.dma_start(out=t, in_=xbf_d[b], transpose=True)
                xT_tiles[b] = t

            ensure_xT(0)

            # ---------------- constants (loaded once) ----------------
            ident = cpool.tile([P, P], BF16, name="ident_sb")
            nc.sync.dma_start(out=ident, in_=id_d[:])
            ap8 = cpool.tile([P, KI, D_IN], F8, name="ap8_sb")
            nc.sync.dma_start(out=ap8, in_=ap8_d[:])
            d18 = cpool.tile([P, KI, S], F8, name="d18_sb")
            nc.sync.dma_start(out=d18, in_=d18_d[:])
            d2t8 = cpool.tile([P, KI, S], F8, name="d2t8_sb")
            nc.sync.dma_start(out=d2t8, in_=d2t8_d[:])
            c0t = cpool.tile([P, KS, S], BF16, name="c0t_sb")
            nc.sync.dma_start(out=c0t, in_=c0t_d[:])
            wv = cpool.tile([P, KI, D_ATT], BF16, name="wv_sb")
            nc.sync.dma_start(out=wv, in_=wv_d[:])
            pbv = cpool.tile([P, KS, D_ATT], BF16, name="pbv_sb")
            nc.sync.dma_start(out=pbv, in_=pbv_d[:])
            wg2 = cpool.tile([P, KD, D_ATT], F8 if fin_fp8 else BF16, name="wg2_sb")
            nc.sync.dma_start(out=wg2, in_=wg2_d[:])
            magic_t = cpool.tile([P, KS], mybir.dt.uint32, name="magic_t")
            nc.vector.memset(magic_t, 0x5F3759DF)
            cf = cpool.tile([P, D_ATT], FP32, name="cf_sb")
            nc.sync.dma_start(out=cf, in_=cf_d[:])
            iota = cpool.tile([P, KS], FP32, name="iota_sb")
            nc.sync.dma_start(out=iota, in_=io_d[:])

            ones8 = cpool.tile([P, KI, 1], F8, name="ones8")
            nc.vector.memset(ones8, 1.0)
            zero_t = cpool.tile([P, 1], FP32, name="zero_t")
            nc.vector.memset(zero_t, 0.0)

            # broadcast mask starts to all 128 partitions on GpSimd
            m_row = cpool.tile([1, n_items], FP32, name="m_row")
            nc.sync.dma_start(out=m_row, in_=m_d[:])
            m_bc = cpool.tile([P, n_items], FP32, name="m_bc")
            nc.gpsimd.partition_broadcast(m_bc, m_row)

            # ---------------- per-item stages ----------------

            def stageA1(b):
                ensure_xT(b)
                ensure_xT(b + 1)
                xT = xT_tiles.pop(b)
                xT8 = wpool.tile([P, KI, S], F8, tag="xT8", name=f"xT8_{b}")
                nc.scalar.copy(xT8, xT)

                # u^T = A'^T x^T (fp8, scaled down by ASCALE on evict);
                # both psum banks evicted in one ACT op
                uT8 = wpool.tile([P, KI, S], F8, tag="uT8", name=f"uT8_{b}")
                ups = psV.tile([P, KI, S], FP32, tag="vpair")
                for e in range(KI):
                    nc.tensor.matmul(
                        ups[:, e, :],
                        lhsT=ap8[:, 0:KI, P * e : P * (e + 1)],
                        rhs=xT8[:, 0:KI, :],
                        start=True, stop=True, perf_mode=DR,
                    )
                nc.scalar.mul(uT8, ups, 1.0 / ASCALE)

                maskb = spool.tile([P, KS], FP32, tag="maskb", name=f"maskb{b}")
                nc.vector.tensor_scalar(
                    maskb, iota, m_bc[:, b : b + 1], NEG, OP.is_ge, OP.mult
                )

                # scoresT k-tiles (fp8 DR + a bf16 identity k-tile adding the
                # constant C0T term) -> exp -> E^T (fp8). Two m-tiles'
                # accumulation groups are emitted interleaved so each
                # LDWEIGHTS overlaps the other group's matmul stream.
                ET8 = wpool.tile([P, KS, S], F8, tag="ET8", name=f"ET8_{b}")
                for mp in range(0, KS, 2):
                    pss = [
                        psW.tile([P, S], FP32, tag="ps", name=f"ps{b}_{mp}_{h}")
                        for h in range(2)
                    ]
                    for lhs_fn, rhs_fn, st, sp in (
                        (lambda m: uT8[:, 0:KI, P * m : P * (m + 1)],
                         lambda m: xT8[:, 0:KI, :], True, False),
                        (lambda m: d2t8[:, 0:KI, P * m : P * (m + 1)],
                         lambda m: xT8[:, 0:KI, :], False, False),
                        (lambda m: xT8[:, 0:KI, P * m : P * (m + 1)],
                         lambda m: d18[:, 0:KI, :], False, False),
                    ):
                        for h in range(2):
                            nc.tensor.matmul(
                                pss[h], lhsT=lhs_fn(mp + h), rhs=rhs_fn(mp + h),
                                start=st, stop=sp, perf_mode=DR,
                            )
                    for h in range(2):
                        nc.tensor.matmul(
                            pss[h], lhsT=ident, rhs=c0t[:, mp + h, :],
                            start=False, stop=True,
                        )
                    for h in range(2):
                        nc.scalar.activation(
                            out=ET8[:, mp + h, :], in_=pss[h], func=AF.Exp,
                            bias=maskb[:, mp + h : mp + h + 1], scale=SCALE,
                        )
                return xT, xT8, ET8

            def stageA2(b, xT, xT8, ET8):
                # V projection (psum keeps pe+bv via identity k-tile);
                # evicted twice: fp8 for the numerator matmul, bf16 for the
                # residual (only one PSUM operand allowed per DVE op).
                # Two m-tiles share a psum bank-pair so each evict is one op.
                V8 = wpool.tile([P, KS, D_ATT], F8, tag="V8", name=f"V8_{b}")
                Vbf = wpool.tile([P, KS, D_ATT], BF16, tag="Vbf", name=f"Vbf_{b}")
                for mp in range(0, KS, 2):
                    vps = psV.tile([P, 2, D_ATT], FP32, tag="vpair")
                    for k in range(KI):
                        for h in range(2):
                            nc.tensor.matmul(
                                vps[:, h, :],
                                lhsT=xT[:, k, P * (mp + h) : P * (mp + h + 1)],
                                rhs=wv[:, k, :],
                                start=(k == 0), stop=False,
                            )
                    for h in range(2):
                        nc.tensor.matmul(
                            vps[:, h, :], lhsT=ident, rhs=pbv[:, mp + h, :],
                            start=False, stop=True,
                        )
                    nc.scalar.copy(V8[:, mp : mp + 2, :], vps)
                    nc.vector.tensor_copy(Vbf[:, mp : mp + 2, :], vps)

                # numerator/denominator + residual; layernorm stats
                den4 = spool.tile([P, KS], FP32, tag="den4", name=f"den4_{b}")
                o4 = wpool.tile([P, KS, D_ATT], BF16, tag="o4", name=f"o4_{b}")
                mv4 = spool.tile([P, KS, 2], FP32, tag="mv4", name=f"mv4_{b}")
                for m in range(KS):
                    nps = psW.tile([P, D_ATT], FP32, tag="ps")
                    dps = psD.tile([P, 1], FP32, tag="dps")
                    for t in range(0, KS, 2):
                        nc.tensor.matmul(
                            nps, lhsT=ET8[:, t : t + 2, P * m : P * (m + 1)],
                            rhs=V8[:, t : t + 2, :],
                            start=(t == 0), stop=(t == KS - 2), perf_mode=DR,
                        )
                        nc.tensor.matmul(
                            dps, lhsT=ET8[:, t : t + 2, P * m : P * (m + 1)],
                            rhs=ones8[:, 0:KI, :],
                            start=(t == 0), stop=(t == KS - 2), perf_mode=DR,
                        )
                    nc.vector.tensor_copy(den4[:, m : m + 1], dps)
                    nc.vector.scalar_tensor_tensor(
                        out=o4[:, m, :], in0=Vbf[:, m, :],
                        scalar=den4[:, m : m + 1], in1=nps,
                        op0=OP.mult, op1=OP.add,
                    )
                    stats = spool.tile([P, 6], FP32, tag="stats")
                    nc.vector.bn_stats(stats, o4[:, m, :])
                    nc.vector.bn_aggr(mv4[:, m, :], stats)

                # batched LN scalars; rsqrt via the fast-inverse-sqrt bit
                # trick + one Newton step, all on DVE - the ACT engine then
                # only ever runs Exp/Identity/Copy (one table set, no
                # ACT_TABLE_LOAD swaps). The 0.25 t2-prescale for the fp8
                # final matmul is folded into the Newton constants for free.
                sq4 = spool.tile([P, KS], FP32, tag="sq4", name=f"sq4_{b}")
                nc.vector.tensor_tensor(sq4, den4, den4, op=OP.mult)
                arg4 = spool.tile([P, KS], FP32, tag="arg4", name=f"arg4_{b}")
                nc.vector.scalar_tensor_tensor(
                    out=arg4, in0=sq4, scalar=EPS, in1=mv4[:, :, 1],
                    op0=OP.mult, op1=OP.add,
                )
                h4 = spool.tile([P, KS], mybir.dt.uint32, tag="h4", name=f"h4_{b}")
                nc.vector.tensor_scalar(
                    h4, arg4.bitcast(mybir.dt.uint32), 1, None,
                    OP.logical_shift_right,
                )
                x0u = spool.tile([P, KS], mybir.dt.uint32, tag="x0u", name=f"x0u_{b}")
                nc.vector.tensor_tensor(x0u, magic_t, h4, op=OP.subtract)
                x0 = x0u.bitcast(FP32)
                tt4 = spool.tile([P, KS], FP32, tag="tt4", name=f"tt4_{b}")
                nc.vector.tensor_tensor(tt4, x0, x0, op=OP.mult)
                nc.vector.tensor_tensor(tt4, tt4, arg4, op=OP.mult)
                u4 = spool.tile([P, KS], FP32, tag="u4", name=f"u4_{b}")
                t2s = 0.25 if fin_fp8 else 1.0
                nc.vector.tensor_scalar(
                    u4, tt4, -0.5 * t2s, 1.5 * t2s, OP.mult, OP.add
                )
                rs4 = spool.tile([P, KS], FP32, tag="rs4", name=f"rs4_{b}")
                nc.vector.tensor_tensor(rs4, u4, x0, op=OP.mult)
                nmr4 = spool.tile([P, KS], FP32, tag="nmr4", name=f"nmr4_{b}")
                nc.vector.scalar_tensor_tensor(
                    out=nmr4, in0=mv4[:, :, 0], scalar=-1.0, in1=rs4,
                    op0=OP.mult, op1=OP.mult,
                )

                t2 = wpool.tile([P, KS, D_ATT], BF16, tag="t2", name=f"t2_{b}")
                for m in range(KS):
                    nc.scalar.activation(
                        out=t2[:, m, :], in_=o4[:, m, :], func=AF.Identity,
                        bias=nmr4[:, m : m + 1], scale=rs4[:, m : m + 1],
                    )

                # y^T via DRAM roundtrip + XBAR (interleaved rows, wg2 matches)
                t2d = dpool.tile([S, D_ATT], BF16, tag="t2d", name=f"t2d_{b}")
                nc.sync.dma_start(
                    out=t2d[:].rearrange("(m p) d -> p m d", p=P), in_=t2
                )
                t2T = wpool.tile([P, KD, S], BF16, tag="t2T", name=f"t2T_{b}")
                nc.sync.dma_start(out=t2T, in_=t2d[:], transpose=True)
                if fin_fp8:
                    t2T8 = wpool.tile([P, KD, S], F8, tag="t2T8", name=f"t2T8_{b}")
                    nc.scalar.copy(t2T8, t2T)
                    return t2T8, t2
                return t2T, t2

            def stageB(b, t2T, t2):
                out_sb = opool.tile([P, KS, D_ATT], FP32, tag="osb")
                for mp in range(0, KS, 2):
                    fps = psV.tile([P, 2, D_ATT], FP32, tag="vpair")
                    for h in range(2):
                        m = mp + h
                        if fin_fp8:
                            for t in range(0, KD, 2):
                                nc.tensor.matmul(
                                    fps[:, h, :],
                                    lhsT=t2T[:, t : t + 2, P * m : P * (m + 1)],
                                    rhs=wg2[:, t : t + 2, :],
                                    start=(t == 0), stop=(t == KD - 2),
                                    perf_mode=DR,
                                )
                        else:
                            for t in range(KD):
                                nc.tensor.matmul(
                                    fps[:, h, :], lhsT=t2T[:, t, P * m : P * (m + 1)],
                                    rhs=wg2[:, t, :],
                                    start=(t == 0), stop=(t == KD - 1),
                                )
                    if fin_fp8:
                        # residual g0*y + evict: y = 4*t2 (res_scale = 4*g0)
                        nc.vector.scalar_tensor_tensor(
                            out=out_sb[:, mp : mp + 2, :], in0=t2[:, mp : mp + 2, :],
                            scalar=float(res_scale), in1=fps,
                            op0=OP.mult, op1=OP.add,
                        )
                    elif has_cf:
                        for h in range(2):
                            nc.vector.tensor_add(
                                out_sb[:, mp + h, :], fps[:, h, :], cf
                            )
                    else:
                        nc.vector.tensor_copy(out_sb[:, mp : mp + 2, :], fps)
                nc.sync.dma_start(
                    out=out_d[b].rearrange("(m p) d -> p m d", p=P), in_=out_sb
                )

            # stageB runs two items behind: the t2 -> DRAM -> XBAR -> fp8
            # roundtrip takes ~10us, so the final matmul of item b-2 is the
            # only stage whose operand is guaranteed ready when the PE gets
            # to it.
            held = {}
            heldA = {}
            for b in range(n_items + 2):
                if b < n_items:
                    heldA[b] = stageA1(b)
                if b >= 2:
                    stageB(b - 2, *held.pop(b - 2))
                if b < n_items:
                    held[b] = stageA2(b, *heldA.pop(b))
    nc.compile()
    return nc


def host_consts(Wq, bq, Wk, bk, Wv, bv, Wf, bf, pos_emb, gamma, beta):
    """One-time host-side weight-layout transforms (input-data independent)."""
    import ml_dtypes

    f32 = np.float32
    bf16 = ml_dtypes.bfloat16
    fp8 = ml_dtypes.float8_e4m3

    Wq = np.asarray(Wq, f32)
    Wk = np.asarray(Wk, f32)
    Wv = np.asarray(Wv, f32)
    Wf = np.asarray(Wf, f32)
    pe = np.asarray(pos_emb, f32)[:S]
    gamma = np.asarray(gamma, f32)
    beta = np.asarray(beta, f32)
    Pq = pe + np.asarray(bq, f32)[None, :]
    Pk = pe + np.asarray(bk, f32)[None, :]
    Pv = pe + np.asarray(bv, f32)[None, :]

    # scores expansion constants (see module docstring)
    Ap = (Wk @ Wq.T) * ASCALE                     # [D_IN, D_IN]
    d1 = Wk @ Pq.T                                # [D_IN, S]
    d2t = Wq @ Pk.T                               # [D_IN, S]
    c0t = Pk @ Pq.T                               # [S, S]

    # blocked row order (128k+p) matching the XBAR transpose output of x^T
    def rows2(a):  # [256, n] -> [128, 2, n]
        return np.ascontiguousarray(a.reshape(KI, P, -1).transpose(1, 0, 2))

    ap_i = Ap.reshape(KI, P, D_IN).transpose(1, 0, 2)

    wg2 = gamma[:, None] * Wf + np.diag(gamma).astype(f32)
    c_row = beta @ Wf + np.asarray(bf, f32) + beta

    return {
        "ap8": np.ascontiguousarray(ap_i).astype(fp8),
        "d18": rows2(d1).astype(fp8),
        "d2t8": rows2(d2t).astype(fp8),
        "c0tb": np.ascontiguousarray(
            c0t.reshape(KS, P, S).transpose(1, 0, 2)
        ).astype(bf16),
        "wvi": rows2(Wv).astype(bf16),
        "pbv": np.ascontiguousarray(
            Pv.reshape(KS, P, D_ATT).transpose(1, 0, 2)
        ).astype(bf16),
        # wg2 rows in the blocked (128k+p) order of the t2T XBAR output
        "wg2i": np.ascontiguousarray(
            wg2.reshape(KD, P, D_ATT).transpose(1, 0, 2)
        ).astype(bf16),
        # fp8 final-matmul variant: 4*g0*Wf (identity split out; t2 = y/4)
        "wg18": np.ascontiguousarray(
            (4.0 * float(gamma[0]) * Wf).reshape(KD, P, D_ATT).transpose(1, 0, 2)
        ).astype(fp8),
        "gamma0": float(gamma[0]),
        "gamma_const": bool(np.allclose(gamma, gamma[0], rtol=0, atol=0)),
        "cfull": np.ascontiguousarray(np.broadcast_to(c_row.astype(f32), (P, D_ATT))),
        "iota4": np.ascontiguousarray(
            (np.arange(P, dtype=f32)[:, None] + P * np.arange(KS, dtype=f32)[None, :])
        ),
        "ident": np.eye(P, dtype=f32).astype(bf16),
    }


_prog_cache = {}


def _get_program(n_items, has_cf, fin_fp8, res_scale):
    key = (n_items, has_cf, fin_fp8, res_scale)
    if key not in _prog_cache:
        _prog_cache[key] = build_program(
            n_items, has_cf=has_cf, fin_fp8=fin_fp8, res_scale=res_scale
        )
    return _prog_cache[key]


def kernel(x, mask_start, Wq, bq, Wk, bk, Wv, bv, Wf, bf, pos_emb, gamma, beta):
    global LAST_RESULTS
    import ml_dtypes

    x_bf = np.asarray(x, np.float32).astype(ml_dtypes.bfloat16)
    mask_f = np.asarray(mask_start).astype(np.float32)
    consts = host_consts(Wq, bq, Wk, bk, Wv, bv, Wf, bf, pos_emb, gamma, beta)

    has_cf = bool(np.abs(consts["cfull"]).max() > 1e-8)
    gamma_const = consts.pop("gamma_const")
    gamma0 = consts.pop("gamma0")
    fin_fp8 = (not has_cf) and gamma_const
    res_scale = 4.0 * gamma0
    if fin_fp8:
        consts.pop("wg2i")
    else:
        consts.pop("wg18")
    nc = _get_program(BPC, has_cf, fin_fp8, res_scale)
    in_maps = []
    for c in range(N_CORES):
        m = dict(consts)
        m["xbf"] = np.ascontiguousarray(x_bf[c * BPC : (c + 1) * BPC])
        m["mstart"] = np.ascontiguousarray(mask_f[c * BPC : (c + 1) * BPC])[None, :]
        in_maps.append(m)

    res = run_bass_kernel_spmd(nc, in_maps, core_ids=list(range(N_CORES)), trace=TRACE)
    LAST_RESULTS = res
    out = np.concatenate([res.results[c]["out"] for c in range(N_CORES)], axis=0)
    return out
